# revision 13
# baseline (speedup 1.0000x reference)
"""UniversalLinear (BitNet b1.58 mode) Trainium2 kernel.

y = Q_int8(RMSNorm(x)) @ Q_ternary(W)^T

Math notes driving the implementation:
  - RMSNorm scale rms_t cancels inside the activation quant:
      x_norm/q = (rms*x) / (rms*m/127) = x * 127/m      (m = absmax per token)
    so the device only needs per-token absmax (m) and sum-of-squares (for the
    final output scale), never a normalized copy of x.
  - x_q in [-127,127] and w_q in {-1,0,1} are exact in fp16, and all partial
    dot products are integers < 2^18, so an fp16 matmul with fp32 PSUM
    accumulation is exact integer arithmetic.
  - Rounding: fp16(x*inv + 1536) == 1536 + round_half_even(x*inv) for
    |x*inv| <= 127 (fp16 has 10 mantissa bits, so ulp == 1 in [1024,2048)).
    The +1536 offset is subtracted after the transpose in a cheap fp16 op.
  - Per-token output scale: os_t = (m_t/127) * w_scale / sqrt(mean(x^2)+eps),
    applied to the PSUM result.
  - Ternary weight quant is per-tensor and tiny: done on the host, shipped as
    a pre-transposed fp16 [128, 4, 512] tensor.

Sharding: pure data-parallel over batch; each of the 8 cores processes
B/8 = 2 batches = 8192 tokens. No collectives.
"""

import numpy as np
import ml_dtypes

import concourse.bass as bass
import concourse.bacc as bacc
import concourse.mybir as mybir
import concourse.tile as tile
from concourse.bass_utils import run_bass_kernel_spmd

N_CORES = 8
B, S, D = 16, 4096, 512
TOK_PER_CORE = (B // N_CORES) * S          # 8192
N_TILES = TOK_PER_CORE // 128              # 64
KC = D // 128                              # 4 contraction chunks
EPS = 1e-8
MAGIC = 1536.0                             # 1.5 * 2^10: fp16 cast rounds to integer

F32 = mybir.dt.float32
BF16 = mybir.dt.float16  # 16-bit matmul dtype (fp16: ints<=2048 exact)
Alu = mybir.AluOpType
Act = mybir.ActivationFunctionType


def build_bass(n_tiles: int = N_TILES) -> bass.Bass:
    nc = bacc.Bacc("TRN2", target_bir_lowering=False, debug=False,
                   num_devices=N_CORES)

    x_d = nc.dram_tensor("x", [n_tiles, 128, D], F32, kind="ExternalInput")
    wqt_d = nc.dram_tensor("wqt", [128, KC, D], BF16, kind="ExternalInput")
    wsb_d = nc.dram_tensor("wsb", [128, 1], F32, kind="ExternalInput")
    y_d = nc.dram_tensor("y", [n_tiles, 128, D], F32, kind="ExternalOutput")

    with tile.TileContext(nc) as tc:
        with (
            tc.tile_pool(name="const", bufs=1) as constp,
            tc.tile_pool(name="io", bufs=6) as iop,
            tc.tile_pool(name="work", bufs=4) as workp,
            tc.tile_pool(name="stats", bufs=8) as statp,
            tc.tile_pool(name="psum", bufs=6, space="PSUM") as psump,
        ):
            wqt = constp.tile([128, KC, D], BF16)
            nc.sync.dma_start(wqt[:], wqt_d[:])
            wsb = constp.tile([128, 1], F32)
            nc.sync.dma_start(wsb[:], wsb_d[:])
            eps_t = constp.tile([128, 1], F32)
            nc.gpsimd.memset(eps_t[:], EPS)

            # Software-pipelined emission: per tick, deepest stage first so
            # each in-order engine stream never head-of-line blocks on a
            # younger tile's long dependency chain.
            T = [dict() for _ in range(n_tiles)]

            def s0_load(t):
                t["xt"] = iop.tile([128, D], F32, tag="xt", name=f"xt{t['i']}")
                nc.sync.dma_start(t["xt"][:], x_d[t["i"]])

            def s1_stats(t):
                xt = t["xt"]
                sq = workp.tile([128, D], F32, tag="sq")
                msq = statp.tile([128, 1], F32, tag="msq")
                nc.scalar.activation(sq[:], xt[:], Act.Square,
                                     scale=float(1.0 / np.sqrt(512.0)),
                                     accum_out=msq[:])
                std = statp.tile([128, 1], F32, tag="std")
                nc.scalar.activation(std[:], msq[:], Act.Sqrt, bias=eps_t[:])
                m = statp.tile([128, 1], F32, tag="m")
                nc.vector.tensor_reduce(m[:], xt[:], axis=mybir.AxisListType.X,
                                        op=Alu.max, apply_absolute_value=True)
                xs = statp.tile([128, 1], F32, tag="xs")      # q = m/127
                nc.vector.tensor_scalar(xs[:], m[:], float(1.0 / 127.0), None,
                                        Alu.mult)
                t["inv"] = statp.tile([128, 1], F32, tag="inv", name=f"inv{t['i']}")
                nc.vector.reciprocal(t["inv"][:], xs[:])
                rstd = statp.tile([128, 1], F32, tag="rstd")
                nc.vector.reciprocal(rstd[:], std[:])
                t1 = statp.tile([128, 1], F32, tag="t1")      # q * w_scale
                nc.vector.tensor_scalar(t1[:], xs[:], wsb[:], None, Alu.mult)
                t["os"] = statp.tile([128, 1], F32, tag="os", name=f"os{t['i']}")  # q*ws/std
                nc.vector.tensor_tensor(t["os"][:], t1[:], rstd[:], Alu.mult)

            def s2_quant(t):
                # fp16(x*inv + 1536) = xq + 1536 exactly (GPSIMD)
                t["xq"] = workp.tile([128, D], BF16, tag="xq", name=f"xq{t['i']}")
                nc.gpsimd.tensor_scalar(t["xq"][:], t["xt"][:], t["inv"][:],
                                        MAGIC, Alu.mult, Alu.add)

            def s3_transpose(t):
                t["xqT"] = workp.tile([128, KC, 128], BF16, tag="xqT", name=f"xqT{t['i']}")
                nc.sync.dma_start(t["xqT"][:], t["xq"][:], transpose=True)

            def s4_fixup(t):
                t["xqTf"] = workp.tile([128, KC, 128], BF16, tag="xqTf", name=f"xqTf{t['i']}")
                nc.vector.tensor_scalar(t["xqTf"][:], t["xqT"][:], MAGIC, None,
                                        Alu.subtract)

            def s5_matmul(t):
                t["ps"] = psump.tile([128, D], F32, tag="ps", name=f"ps{t['i']}")
                for j in range(KC):
                    nc.tensor.matmul(t["ps"][:], t["xqTf"][:, j, :],
                                     wqt[:, j, :],
                                     start=(j == 0), stop=(j == KC - 1))

            def s6_store(t):
                yt = iop.tile([128, D], F32, tag="yt")
                nc.vector.tensor_scalar(yt[:], t["ps"][:], t["os"][:], None,
                                        Alu.mult)
                nc.sync.dma_start(y_d[t["i"]], yt[:])

            stages = [s6_store, s5_matmul, s4_fixup, s3_transpose,
                      s2_quant, s1_stats, s0_load]
            n_st = len(stages)
            for i in range(n_tiles):
                T[i]["i"] = i
            for tick in range(n_tiles + n_st - 1):
                for depth, fn in enumerate(stages):
                    i = tick - (n_st - 1 - depth)
                    if 0 <= i < n_tiles:
                        fn(T[i])

    nc.compile()
    return nc


def host_prep(weight: np.ndarray, norm_weight: np.ndarray):
    """Quantize the weight on the host (exact ternary + per-tensor scale)."""
    w = weight.astype(np.float64)
    ws = max(float(np.mean(np.abs(w))), EPS)
    wq = np.round(np.clip(w / ws, -1.0, 1.0))          # {-1, 0, +1}
    # pre-transposed chunks: wqt[p, j, o] = wq[o, j*128 + p]
    wqt = np.ascontiguousarray(
        wq.T.reshape(KC, 128, D).transpose(1, 0, 2)
    ).astype(np.float16)
    wsb = np.full((128, 1), np.float32(ws), dtype=np.float32)
    return wqt, wsb


_NC_CACHE: dict[int, bass.Bass] = {}


def _get_nc(n_tiles: int = N_TILES) -> bass.Bass:
    if n_tiles not in _NC_CACHE:
        _NC_CACHE[n_tiles] = build_bass(n_tiles)
    return _NC_CACHE[n_tiles]


def _run(x: np.ndarray, weight: np.ndarray, norm_weight: np.ndarray,
         trace: bool = False):
    wqt, wsb = host_prep(weight, norm_weight)
    nc = _get_nc()
    shards = x.reshape(N_CORES, N_TILES, 128, D)
    in_maps = [
        {"x": np.ascontiguousarray(shards[c]), "wqt": wqt, "wsb": wsb}
        for c in range(N_CORES)
    ]
    res = run_bass_kernel_spmd(nc, in_maps, list(range(N_CORES)), trace=trace)
    y = np.stack([res.results[c]["y"] for c in range(N_CORES)])
    return y.reshape(B, S, D).astype(np.float32, copy=False), res


def _reference_host(x, weight, norm_weight):
    # numpy fallback, only used if norm_weight is not all-ones
    x = x.astype(np.float32)
    rms = 1.0 / np.sqrt(np.mean(x * x, axis=-1, keepdims=True) + EPS)
    xn = x * rms * norm_weight.astype(np.float32)
    sc = np.maximum(np.max(np.abs(xn), axis=-1, keepdims=True), EPS) / 127.0
    xdq = np.round(np.clip(xn / sc, -128.0, 127.0)) * sc
    w = weight.astype(np.float32)
    ws = np.maximum(np.mean(np.abs(w)), EPS)
    wdq = np.round(np.clip(w / ws, -1.0, 1.0)) * ws
    return (xdq.reshape(-1, D) @ wdq.T).reshape(x.shape[:-1] + (D,))


def kernel(x: np.ndarray, weight: np.ndarray,
           norm_weight: np.ndarray) -> np.ndarray:
    if not np.all(norm_weight == 1.0):
        return _reference_host(x, weight, norm_weight).astype(np.float32)
    y, _ = _run(np.asarray(x, dtype=np.float32),
                np.asarray(weight, dtype=np.float32),
                np.asarray(norm_weight, dtype=np.float32))
    return y
